# revision 93
# baseline (speedup 1.0000x reference)
"""Trainium2 Bass kernel for GCMC-style GNN message passing (nn_Net_6425271075083).

Strategy (8 NeuronCores, users sharded 1250/core):
  - Host converts the edge lists into dense per-rating adjacency count
    matrices in BOTH layouts (counts are small ints, EXACT in fp8e4 ->
    half the HBM traffic of bf16 at full bf16 matmul speed, since the PE
    allows mixed bf16 x fp8 operands) and the implicit-feedback index
    matrix into a per-user histogram; degrees -> cu/ci norms.
  - All constants are packed host-side into a few [128, N] blobs, loaded
    with a handful of large HWDGE DMAs ordered so each phase's operands
    arrive just ahead of first use (W[r=0]+ufT gate the first matmul).
  - Device dense math per core, scheduled to keep PE >80% busy:
      item side:  M_c^T = sum_r (D_cu (ufeat_c @ W_r))^T @ A_r[users_c]
                  (bf16 x fp8) -> fp16 AllReduce -> item_agg^T
      user side:  hi = D_ci (ifeat @ W_r); user_agg_m = sum_{r,k}
                  A^T-blocks(fp8) @ hi; Lrelu+cu fused on ACT engine.
      heads:      hist/Y0 matmuls (fp16) open the s^T PSUM groups early;
                  fc_w terms close them per 512-user chunk as soon as its
                  PE transposes land; q^T is computed mid-user-phase right
                  after the AllReduce; row 64 of q'/s' carries 1/ci and
                  bu+gm (fp16 keeps the big 1/ci products accurate).
      final:      out_c = D_ci (q'^T.T @ s'^T) + bi in fp16, emitted
                  per user-chunk interleaved INTO the user loop; evicts
                  alternate DVE/ACT; one strided DMA per chunk.
  - Elementwise evictions alternate DVE/ACT so neither queue gates PE;
    the Lrelu act-table is pre-warmed during the startup DMAs.
  - The last output chunk ships as two half-band DMAs so the transfer
    overlaps the remaining PSUM evictions (shorter pipeline drain).
  - HW-measured max error ~7.3e-3 scale-relative (rel-L2 1.4e-3) vs the
    fp32 reference (tolerance 2e-2); ~127us/core HW vs 216us baseline
    (TimelineSim 136.3us single-core; PE ~87% occupied vs a ~117us
    matmul floor at bf16). PSUM banks are budgeted per phase: item
    psx4+psB4; hi psS2(rotating)+psh6; user psS2+psu2+psT2+psO2.
"""
import numpy as np
import ml_dtypes

import concourse.bass as bass
import concourse.bacc as bacc
import concourse.mybir as mybir
import concourse.tile as tile
from concourse import bass_utils
from concourse.masks import make_identity

BF = ml_dtypes.bfloat16
F8 = ml_dtypes.float8_e4m3
F32 = mybir.dt.float32
F16 = mybir.dt.float16
BF16 = mybir.dt.bfloat16
FP8 = mybir.dt.float8e4

N_CORES = 8
U, I, R, D, O, H = 10000, 1000, 5, 256, 64, 1001
UC = U // N_CORES          # 1250
UCP = 1280                 # users per core, padded
IP = 1024                  # items padded
HP = 1024                  # hist bins padded
KU = UCP // 128            # 10 user k/m tiles
KI = IP // 128             # 8 item k/m tiles
KH = HP // 128             # 8 hist k tiles
UCHUNKS = [(0, 512), (512, 512), (1024, 226)]   # user free-dim chunks (valid)
ICHUNKS = [(0, 512), (512, 488)]                # item free-dim chunks (valid)

# blob_w0 (bf16): W[r=0] tiles; blob_w1: W[r=1..4]; blob_u0/u1: ufT halves
A_WIDTH0 = 512
A_WIDTH1 = 2048
# blob_b (bf16): ifT tiles then fc_w tiles
B_IF = 0          # ift[kk] at kk*1024
B_FCW = 2048      # fcw[k] at B_FCW + k*64
B_WIDTH = 2176
# blob_c (bf16): Y0 tiles then hist tiles
C_Y0 = 0          # y0[k] at k*64
C_HIST = 512      # hist[k] at C_HIST + k*1280
C_WIDTH = 512 + KH * UCP
# fblob (f32) columns
F_CU = 0          # 10 cols
F_CI = 10         # 8 cols
F_BI = 18         # 8 cols
F_FCB = 26        # 1 col (partitions 0..63)
F_FCW = 27        # fc_w as f32, 2*O cols; device casts to fp16 once
F_WIDTH = 27 + 2 * O
# rowv (f32, 1 partition)
RV_CIR = 0        # 1/ci row, IP cols
RV_BU = IP        # bu + global_mean row, UCP cols
RV_WIDTH = IP + UCP

_ALU = mybir.AluOpType
_AF = mybir.ActivationFunctionType


def host_preprocess(src_idx, dst_idx, implicit_matrix, sqrt_count, global_mean,
                    ufeat, ifeat, W, fc_w, fc_b, bu, bi, Y):
    """Build per-core input maps (layout/sharding only plus degree/adjacency
    densification; all NN math happens on device)."""
    src = np.asarray(src_idx).astype(np.int64)
    dst = np.asarray(dst_idx).astype(np.int64)
    im = np.asarray(implicit_matrix).astype(np.int64)
    sqrt_count = np.asarray(sqrt_count, np.float32)
    gm = float(np.asarray(global_mean, np.float32).reshape(1)[0])
    ufeat = np.asarray(ufeat, np.float32)
    ifeat = np.asarray(ifeat, np.float32)
    W = np.asarray(W, np.float32)
    fc_w = np.asarray(fc_w, np.float32)
    fc_b = np.asarray(fc_b, np.float32)
    bu = np.asarray(bu, np.float32)
    bi = np.asarray(bi, np.float32)
    Y = np.asarray(Y, np.float32)

    deg_u = np.bincount(src.reshape(-1), minlength=U).astype(np.float32)
    deg_i = np.bincount(dst.reshape(-1), minlength=I).astype(np.float32)
    cu = 1.0 / np.sqrt(np.maximum(deg_u, 1.0))
    ci = 1.0 / np.sqrt(np.maximum(deg_i, 1.0))

    def pack_cols(vec, ntiles):
        padded = np.zeros(128 * ntiles, np.float32)
        padded[:len(vec)] = vec
        return padded.reshape(ntiles, 128).T.copy()

    # dense adjacency counts per rating [U, I]
    G = np.zeros((R, U, I), np.float32)
    for r in range(R):
        G[r] = np.bincount(src[r] * I + dst[r], minlength=U * I).reshape(U, I)
    assert G.max() <= 16, "adjacency counts must stay exact in fp8e4"

    # implicit histogram [U, H] with 1/sqrt_count folded
    hist = np.bincount((np.arange(U)[:, None] * H + im).reshape(-1),
                       minlength=U * H).reshape(U, H).astype(np.float32)
    histp = hist / sqrt_count

    Y0 = Y.copy()
    Y0[0] = 0.0

    fblob_shared = np.zeros((128, F_WIDTH), np.float32)
    fblob_shared[:, F_CI:F_CI + KI] = pack_cols(ci, KI)
    fblob_shared[:, F_BI:F_BI + KI] = pack_cols(bi[:, 0], KI)
    fblob_shared[0:O, F_FCB] = fc_b
    for k in range(2):
        fblob_shared[:, F_FCW + k * O:F_FCW + (k + 1) * O] = fc_w[k * 128:(k + 1) * 128]

    blob_b = np.zeros((128, B_WIDTH), BF)
    ifT = np.zeros((D, IP), np.float32)
    ifT[:, :I] = ifeat.T
    for kk in range(2):
        blob_b[:, B_IF + kk * IP:B_IF + (kk + 1) * IP] = \
            ifT[kk * 128:(kk + 1) * 128].astype(BF)

    in_maps = []
    for c in range(N_CORES):
        us = slice(c * UC, (c + 1) * UC)

        # ga: [R, 128, KU*IP] fp8 — ga[r, p, k*IP+i] = G[r][us][k*128+p, i]
        ga = np.zeros((R, 128, KU * IP), F8)
        # gb: [KU, 128, R*KI*128] fp8 — gb[m, p, r*IP+k*128+u] = G^T blocks
        gb = np.zeros((KU, 128, R * KI * 128), F8)
        for r in range(R):
            Gc = np.zeros((UCP, IP), np.float32)
            Gc[:UC, :I] = G[r][us]
            ga[r] = Gc.reshape(KU, 128, IP).transpose(1, 0, 2) \
                      .reshape(128, KU * IP).astype(F8)
            blocks = Gc.T.reshape(KI, 128, KU, 128).transpose(2, 1, 0, 3)
            gb[:, :, r * IP:(r + 1) * IP] = \
                blocks.reshape(KU, 128, KI * 128).astype(F8)

        blob_w0 = np.zeros((128, A_WIDTH0), BF)
        blob_w1 = np.zeros((128, A_WIDTH1), BF)
        for r in range(R):
            for kk in range(2):
                wt = W[r, kk * 128:(kk + 1) * 128].astype(BF)
                if r == 0:
                    blob_w0[:, kk * D:(kk + 1) * D] = wt
                else:
                    blob_w1[:, ((r - 1) * 2 + kk) * D:((r - 1) * 2 + kk + 1) * D] = wt
        ufT = np.zeros((D, UCP), np.float32)
        ufT[:, :UC] = ufeat[us].T
        blob_u = [ufT[kk * 128:(kk + 1) * 128].astype(BF) for kk in range(2)]

        blob_c = np.zeros((128, C_WIDTH), np.float16)
        y0_t = np.zeros((HP, O), np.float32)
        y0_t[:H] = Y0
        for k in range(KH):
            blob_c[:, C_Y0 + k * O:C_Y0 + (k + 1) * O] = y0_t[k * 128:(k + 1) * 128]
        hist_t = np.zeros((HP, UCP), np.float32)
        hist_t[:H, :UC] = histp[us].T
        for k in range(KH):
            blob_c[:, C_HIST + k * UCP:C_HIST + (k + 1) * UCP] = \
                hist_t[k * 128:(k + 1) * 128]

        fblob = fblob_shared.copy()
        fblob[:, F_CU:F_CU + KU] = pack_cols(cu[us], KU)

        rowv = np.zeros((1, RV_WIDTH), np.float32)
        rowv[0, RV_CIR:RV_CIR + I] = 1.0 / ci
        rowv[0, RV_BU:RV_BU + UC] = bu[us, 0] + gm

        in_maps.append({
            "ga": ga, "gb": gb,
            "blob_w0": blob_w0, "blob_w1": blob_w1,
            "blob_u0": blob_u[0], "blob_u1": blob_u[1],
            "blob_b": blob_b, "blob_c": blob_c,
            "fblob": fblob, "rowv": rowv,
        })
    return in_maps


def declare_io(nc, timing_mode=False):
    t = {}
    def inp(name, shape, dt):
        t[name] = nc.dram_tensor(name, list(shape), dt, kind="ExternalInput").ap()
    inp("ga", (R, 128, KU * IP), FP8)
    inp("gb", (KU, 128, R * KI * 128), FP8)
    inp("blob_w0", (128, A_WIDTH0), BF16)
    inp("blob_w1", (128, A_WIDTH1), BF16)
    inp("blob_u0", (128, UCP), BF16)
    inp("blob_u1", (128, UCP), BF16)
    inp("blob_b", (128, B_WIDTH), BF16)
    inp("blob_c", (128, C_WIDTH), F16)
    inp("fblob", (128, F_WIDTH), F32)
    inp("rowv", (1, RV_WIDTH), F32)
    if timing_mode:
        t["tick"] = nc.dram_tensor("tick", [1, 4], F16, kind="ExternalOutput").ap()
    else:
        t["out"] = nc.dram_tensor("out", [IP, UC], F16, kind="ExternalOutput").ap()
    return t


def emit_body(nc, tc, t, it, timing_mode=False, loop_mode=False):
    """Emit one full compute pass. `it` suffixes tile names for repeats."""
    from contextlib import ExitStack
    ctx = ExitStack()
    P = 128

    const = ctx.enter_context(tc.tile_pool(name=f"const{it}", bufs=1))

    ident = const.tile([P, P], F32, name=f"ident{it}")
    make_identity(nc, ident[:])
    identb = const.tile([P, P], F16, name=f"identb{it}")
    nc.vector.tensor_copy(identb[:], ident[:])
    fcw16 = const.tile([P, 2 * O], F16, name=f"fcw16{it}")

    # DMA order matters: blob_a gates the first matmul, ga gates the item
    # phase; hist (blob_c) and gb are consumed much later. The tiny f32
    # tensors ride the scalar-engine HWDGE queue so they don't delay blob_a.
    blob_w0 = const.tile([P, A_WIDTH0], BF16, name=f"blob_w0{it}")
    nc.sync.dma_start(blob_w0[:], t["blob_w0"][:])
    blob_u = []
    for kk in range(2):
        bu_t = const.tile([P, UCP], BF16, name=f"blob_u{kk}{it}")
        nc.sync.dma_start(bu_t[:], t[f"blob_u{kk}"][:])
        blob_u.append(bu_t)
    fblob = const.tile([P, F_WIDTH], F32, name=f"fblob{it}")
    nc.scalar.dma_start(fblob[:], t["fblob"][:])
    ga_pool = ctx.enter_context(tc.tile_pool(name=f"gap{it}", bufs=3))
    blob_w1 = const.tile([P, A_WIDTH1], BF16, name=f"blob_w1{it}")
    ga_t = []
    for r in range(R):
        g = ga_pool.tile([P, KU * IP], FP8, name=f"ga{r}{it}", tag="ga")
        nc.sync.dma_start(g[:], t["ga"][r])
        ga_t.append(g)
        if r == 0:
            nc.sync.dma_start(blob_w1[:], t["blob_w1"][:])
    rowv = const.tile([1, RV_WIDTH], F32, name=f"rowv{it}")
    nc.scalar.dma_start(rowv[:], t["rowv"][:])
    blob_b = const.tile([P, B_WIDTH], BF16, name=f"blob_b{it}")
    nc.sync.dma_start(blob_b[:], t["blob_b"][:])
    blob_c = const.tile([P, C_WIDTH], F16, name=f"blob_c{it}")
    nc.sync.dma_start(blob_c[:], t["blob_c"][:])
    gb_pool = ctx.enter_context(tc.tile_pool(name=f"gbp{it}", bufs=6))
    gb_t = []
    for m in range(KU):
        g = gb_pool.tile([P, R * KI * P], FP8, name=f"gb{m}{it}", tag="gb")
        nc.sync.dma_start(g[:], t["gb"][m])
        gb_t.append(g)

    nc.vector.tensor_copy(fcw16[:], fblob[:, F_FCW:F_FCW + 2 * O])
    # warm the Lrelu+Identity activation-function set during the startup DMA
    # wait so no LoadActFuncSet lands on the critical path later
    actwarm = const.tile([1, 8], F32, name=f"actwarm{it}")
    nc.scalar.activation(actwarm[:], ident[0:1, 0:8], _AF.Lrelu, alpha=0.1)

    def wsl(r, kk):
        if r == 0:
            return blob_w0[:, kk * D:(kk + 1) * D]
        return blob_w1[:, ((r - 1) * 2 + kk) * D:((r - 1) * 2 + kk + 1) * D]

    def ufsl(kk, k):
        return blob_u[kk][:, k * P:(k + 1) * P]

    def ifsl(kk, k):
        c = B_IF + kk * IP + k * P
        return blob_b[:, c:c + P]

    # ---------------- item phase ----------------
    from contextlib import ExitStack as _ES
    xw_pool = ctx.enter_context(tc.tile_pool(name=f"xw{it}", bufs=6))
    item_ctx = _ES()
    psx_pool = item_ctx.enter_context(tc.tile_pool(name=f"psx{it}", bufs=4, space="PSUM"))
    psb_pool = item_ctx.enter_context(tc.tile_pool(name=f"psb{it}", bufs=1, space="PSUM"))

    psB = [[psb_pool.tile([P, 512], F32, name=f"psB{h}{cix}{it}")
            for cix in range(2)] for h in range(2)]
    n_rk = R * KU
    rk = 0
    for r in range(R):
        for k in range(KU):
            psx = psx_pool.tile([P, D], F32, name=f"psx{it}")
            for kk in range(2):
                nc.tensor.matmul(psx[:], ufsl(kk, k), wsl(r, kk),
                                 start=(kk == 0), stop=(kk == 1))
            xh = xw_pool.tile([P, D], BF16, name=f"xh{it}", tag="xh")
            if rk % 2 == 0:
                nc.scalar.activation(xh[:], psx[:], _AF.Identity,
                                     scale=fblob[:, F_CU + k:F_CU + k + 1])
            else:
                nc.vector.tensor_scalar_mul(xh[:], psx[:],
                                            fblob[:, F_CU + k:F_CU + k + 1])
            for h in range(2):
                for cix, (c0, cw) in enumerate(ICHUNKS):
                    nc.tensor.matmul(
                        psB[h][cix][:, 0:cw], xh[:, h * P:(h + 1) * P],
                        ga_t[r][:, k * IP + c0:k * IP + c0 + cw],
                        start=(rk == 0), stop=(rk == n_rk - 1))
            rk += 1

    # both 128-row halves of M_c^T live in one SBUF tile; the AllReduce runs
    # SBUF->SBUF, skipping the DRAM round-trip entirely
    mcTc = const.tile([P, 2 * I], F16, name=f"mcTc{it}")
    iagc = const.tile([P, 2 * I], F16, name=f"iagc{it}")
    for h in range(2):
        for cix, (c0, cw) in enumerate(ICHUNKS):
            if (h + cix) % 2 == 0:
                nc.vector.tensor_copy(mcTc[:, h * I + c0:h * I + c0 + cw],
                                      psB[h][cix][:, 0:cw])
            else:
                nc.scalar.activation(mcTc[:, h * I + c0:h * I + c0 + cw],
                                     psB[h][cix][:, 0:cw], _AF.Identity)

    dramc = ctx.enter_context(tc.tile_pool(name=f"dramc{it}", bufs=1, space="DRAM"))
    itemp = dramc.tile([P, 2 * I], F16, name=f"itemp{it}")
    itemagg = dramc.tile([P, 2 * I], F16, name=f"itemagg{it}",
                         addr_space="Local" if loop_mode else "Shared")
    nc.scalar.dma_start(itemp[:], mcTc[:])
    if loop_mode:
        # collectives can't live inside control flow; equivalent-size DMA copy
        nc.gpsimd.dma_start(itemagg[:], itemp[:])
    else:
        nc.gpsimd.collective_compute(
            "AllReduce", _ALU.add,
            replica_groups=[list(range(N_CORES))],
            ins=[itemp.opt()], outs=[itemagg.opt()],
        )
    nc.scalar.dma_start(iagc[:], itemagg[:])
    item_ctx.close()

    # ---------------- user phase: hi (+ y-head matmuls into psS) ----------------
    psy_pool = ctx.enter_context(tc.tile_pool(name=f"psy{it}", bufs=2, space="PSUM"))
    hi_pool = ctx.enter_context(tc.tile_pool(name=f"hi{it}", bufs=R * KI))
    hi_ctx = _ES()
    psh_pool = hi_ctx.enter_context(tc.tile_pool(name=f"psh{it}", bufs=6, space="PSUM"))
    # psS accumulation groups are opened here with the hist/Y0 matmuls (they
    # depend only on blob_c); the fc_w terms land in the tail with stop=True.
    psS = [psy_pool.tile([O, 512], F32, name=f"psS{c}{it}", tag="psS") for c in range(3)]

    def emit_y_head(ci_):
        c0, cw = UCHUNKS[ci_]
        for kh in range(KH):
            nc.tensor.matmul(psS[ci_][:, 0:cw],
                             blob_c[:, C_Y0 + kh * O:C_Y0 + (kh + 1) * O],
                             blob_c[:, C_HIST + kh * UCP + c0:C_HIST + kh * UCP + c0 + cw],
                             start=(kh == 0), stop=False)

    emit_y_head(0)
    hi = {}
    for r in range(R):
        for k in range(KI):
            psh = psh_pool.tile([P, D], F32, name=f"psh{it}")
            for kk in range(2):
                nc.tensor.matmul(psh[:], ifsl(kk, k), wsl(r, kk),
                                 start=(kk == 0), stop=(kk == 1))
            hh = hi_pool.tile([P, D], BF16, name=f"hih{r}_{k}{it}", tag="hi")
            idx = r * KI + k
            if idx >= R * KI - 6:
                # trailing evicts gate the hi_ctx pool close (and with it the
                # user-phase PSUM alloc): split them across both engines
                nc.vector.tensor_scalar_mul(hh[:, 0:128], psh[:, 0:128],
                                            fblob[:, F_CI + k:F_CI + k + 1])
                nc.scalar.activation(hh[:, 128:256], psh[:, 128:256], _AF.Identity,
                                     scale=fblob[:, F_CI + k:F_CI + k + 1])
            elif idx % 2 == 0:
                nc.vector.tensor_scalar_mul(hh[:], psh[:],
                                            fblob[:, F_CI + k:F_CI + k + 1])
            else:
                nc.scalar.activation(hh[:], psh[:], _AF.Identity,
                                     scale=fblob[:, F_CI + k:F_CI + k + 1])
            hi[(r, k)] = hh
    emit_y_head(1)
    emit_y_head(2)
    hi_ctx.close()

    # ---------------- q head (emitted after user m=1; needs AllReduce) -------
    iag_pool = ctx.enter_context(tc.tile_pool(name=f"iag{it}", bufs=2))
    act_pool = ctx.enter_context(tc.tile_pool(name=f"actp{it}", bufs=2))
    head_ctx = _ES()
    pso_pool = head_ctx.enter_context(tc.tile_pool(name=f"pso{it}", bufs=2, space="PSUM"))
    qT = const.tile([O + 1, IP], F16, name=f"qT{it}")
    nc.vector.memset(qT[:, I:IP], 0.0)

    def emit_q_head():
        qacts = []
        for kk in range(2):
            iag = iagc[:, kk * I:(kk + 1) * I]
            qact = iag_pool.tile([P, I], F16, name=f"qact{kk}{it}", tag="qact")
            # halves on alternating engines: nothing blocks a queue >600ns
            for hf, (h0, hw) in enumerate(ICHUNKS):
                if (kk + hf) % 2 == 0:
                    nc.vector.scalar_tensor_tensor(
                        qact[:, h0:h0 + hw], iag[:, h0:h0 + hw], 0.1,
                        iag[:, h0:h0 + hw], _ALU.mult, _ALU.max)
                else:
                    nc.scalar.activation(qact[:, h0:h0 + hw], iag[:, h0:h0 + hw],
                                         _AF.Lrelu, alpha=0.1)
            qacts.append(qact)
        for (c0, cw) in ICHUNKS:
            psQ = pso_pool.tile([O, 512], F32, name=f"psQ{it}", tag="pso")
            for kk in range(2):
                nc.tensor.matmul(psQ[:, 0:cw], fcw16[:, kk * O:(kk + 1) * O],
                                 qacts[kk][:, c0:c0 + cw],
                                 start=(kk == 0), stop=(kk == 1))
            nc.scalar.activation(qT[0:O, c0:c0 + cw], psQ[:, 0:cw], _AF.Identity,
                                 bias=fblob[0:O, F_FCB:F_FCB + 1])
        nc.vector.tensor_copy(qT[O:O + 1, :], rowv[0:1, RV_CIR:RV_CIR + IP])

    # ------- user phase: user_agg + transposes, finals interleaved ---------
    # The final phase for user-chunk c is emitted as soon as its last m-tile
    # is transposed, so it fills PE/ACT/DVE/DMA while later m-tiles compute.
    dram = ctx.enter_context(tc.tile_pool(name=f"dram{it}", bufs=1, space="DRAM"))
    if timing_mode:
        out_dst = dram.tile([IP, UC], F16, name=f"outscratch{it}")
    else:
        out_dst = t["out"]
    out_pool = head_ctx.enter_context(tc.tile_pool(name=f"outp{it}", bufs=1))
    user_ctx = _ES()
    psu_pool = user_ctx.enter_context(tc.tile_pool(name=f"psu{it}", bufs=2, space="PSUM"))
    pst_pool = user_ctx.enter_context(tc.tile_pool(name=f"pst{it}", bufs=2, space="PSUM"))
    # actT is chunked so each final-phase user-chunk only waits for its own
    # transposes (tile-granular deps), overlapping the final with user_agg.
    actT = [[const.tile([P, cw], F16, name=f"actT{j}c{ci_}{it}") for j in range(2)]
            for ci_, (c0, cw) in enumerate(UCHUNKS)]
    last_out_t = [None]

    def emit_final_chunk(ci_):
        c0, cw = UCHUNKS[ci_]
        for kk in range(2):
            nc.tensor.matmul(psS[ci_][:, 0:cw],
                             fcw16[:, kk * O:(kk + 1) * O],
                             actT[ci_][kk][:, 0:cw],
                             start=False, stop=(kk == 1))
        sT = const.tile([O + 1, 512], F16, name=f"sT{ci_}{it}")
        nc.scalar.activation(sT[0:O, 0:cw], psS[ci_][:, 0:cw], _AF.Identity,
                             bias=fblob[0:O, F_FCB:F_FCB + 1])
        nc.vector.tensor_copy(sT[O:O + 1, 0:cw], rowv[0:1, RV_BU + c0:RV_BU + c0 + cw])
        vw = min(cw, max(0, UC - c0))
        # all 8 item-tiles land in one SBUF tile; chunks 0/1 then issue a
        # single strided DMA for the whole [1024, vw] column band (1 HWDGE
        # descriptor-gen instead of 8). The last chunk writes per-mi so the
        # tail drains as soon as each evict lands.
        out_c = out_pool.tile([P, KI * cw], F16, name=f"out_c{ci_}{it}")
        for mi in range(KI):
            # chunk 2 runs after the last psu eviction: its dead (bank-
            # rounded 2KB) slots double the tail's psO pipeline depth
            if ci_ == 2 and mi % 2 == 1:
                psO = psu_pool.tile([P, 512], F32, name=f"psOu{it}", tag="psu")
            else:
                psO = pso_pool.tile([P, 512], F32, name=f"psO{it}", tag="pso")
            nc.tensor.matmul(psO[:, 0:cw], qT[:, mi * P:(mi + 1) * P],
                             sT[:, 0:cw], start=True, stop=True)
            if mi % 2 == 0:
                nc.vector.tensor_scalar(out_c[:, mi * cw:mi * cw + cw], psO[:, 0:cw],
                                        fblob[:, F_CI + mi:F_CI + mi + 1],
                                        fblob[:, F_BI + mi:F_BI + mi + 1],
                                        _ALU.mult, _ALU.add)
            else:
                nc.scalar.activation(out_c[:, mi * cw:mi * cw + cw], psO[:, 0:cw],
                                     _AF.Identity,
                                     bias=fblob[:, F_BI + mi:F_BI + mi + 1],
                                     scale=fblob[:, F_CI + mi:F_CI + mi + 1])
        nmi = 2 if ci_ == 2 else KI
        srcv = out_c[:].rearrange("p (mi c) -> p mi c", mi=KI)
        dstv = out_dst.rearrange("(mi p) u -> p mi u", p=P)
        # the last chunk ships in two halves so the first transfer overlaps
        # the second half's evictions (shorter pipeline drain)
        for b0 in range(0, KI, nmi):
            nc.scalar.dma_start(dstv[0:P, b0:b0 + nmi, c0:c0 + vw],
                                srcv[:, b0:b0 + nmi, 0:vw])
        last_out_t[0] = out_c

    chunk_last_m = {3: 0, 7: 1, KU - 1: 2}
    for m in range(KU):
        psu = psu_pool.tile([P, D], F32, name=f"psu{it}")
        i = 0
        nmm = R * KI
        for r in range(R):
            for k in range(KI):
                nc.tensor.matmul(psu[:], gb_t[m][:, r * IP + k * P:r * IP + (k + 1) * P],
                                 hi[(r, k)][:],
                                 start=(i == 0), stop=(i == nmm - 1))
                i += 1
        act = act_pool.tile([P, D], F16, name=f"act{it}", tag="act")
        nc.scalar.activation(act[:], psu[:], _AF.Lrelu,
                             scale=fblob[:, F_CU + m:F_CU + m + 1], alpha=0.1)
        mc, mo = m // 4, (m % 4) * P
        mcols = min(P, UCHUNKS[mc][1] - mo)
        for j in range(2):
            psT = pst_pool.tile([P, P], F16, name=f"psT{it}")
            nc.tensor.transpose(psT[:], act[:, j * P:(j + 1) * P], identb[:])
            if j == 0:
                nc.vector.tensor_copy(actT[mc][j][:, mo:mo + mcols], psT[0:P, 0:mcols])
            else:
                nc.scalar.activation(actT[mc][j][:, mo:mo + mcols], psT[0:P, 0:mcols],
                                     _AF.Identity)
        if m == 2:
            emit_q_head()
        if m == 4:
            # psS2 rotates into psS0's bank, freed by chunk 0's sT eviction
            emit_y_head(2)
        if m in chunk_last_m:
            emit_final_chunk(chunk_last_m[m])

    user_ctx.close()
    head_ctx.close()
    if timing_mode:
        nc.scalar.dma_start(t["tick"][:], last_out_t[0][0:1, 0:4])
    ctx.close()


_PROGRAM_CACHE = {}


def build_program(repeat=1, timing_mode=False):
    key = (repeat, timing_mode)
    if key in _PROGRAM_CACHE:
        return _PROGRAM_CACHE[key]
    nc = bacc.Bacc("TRN2", target_bir_lowering=False, debug=False,
                   num_devices=N_CORES)
    t = declare_io(nc, timing_mode)
    with tile.TileContext(nc) as tc:
        for it in range(repeat):
            emit_body(nc, tc, t, f"_i{it}" if repeat > 1 else "",
                      timing_mode=timing_mode)
    nc.compile()
    _PROGRAM_CACHE[key] = (nc, t)
    return nc, t


def build_loop_program(trips):
    key = ("loop", trips)
    if key in _PROGRAM_CACHE:
        return _PROGRAM_CACHE[key]
    nc = bacc.Bacc("TRN2", target_bir_lowering=False, debug=False,
                   num_devices=N_CORES)
    t = declare_io(nc, timing_mode=True)
    with tile.TileContext(nc) as tc:
        with tc.For_i(0, trips, 1):
            emit_body(nc, tc, t, "", timing_mode=True, loop_mode=True)
    nc.compile()
    _PROGRAM_CACHE[key] = (nc, t)
    return nc, t


def kernel(**inputs):
    in_maps = host_preprocess(**inputs)
    nc, _ = build_program()
    res = bass_utils.run_bass_kernel_spmd(
        nc, in_maps, core_ids=list(range(N_CORES)), trace=False)
    out = np.concatenate([res.results[c]["out"][:I].astype(np.float32)
                          for c in range(N_CORES)], axis=1)
    return out
